# revision 8
# baseline (speedup 1.0000x reference)
"""GCMC (gnn_message_passing) Trainium2 Bass kernel, 8-core SPMD.

Strategy (hardcoded for the nn_GCMC_40870908789353 shapes):
- Core c owns users [c*6250,(c+1)*6250) and items [c*2500,(c+1)*2500), laid
  out locally as users at rows [0,6250), items at [6272,8772), block 8832.
- Dead-code elimination: scores only read x rows at user_nodes/item_nodes,
  so edges whose destination is unsampled (and word pairs whose item is
  unsampled) are dropped during CPU-side sharding. This is exact.
- The GCN aggregation is linear, so we aggregate normalized embeddings
  first and apply conv_weight after: agg = segsum(xn[src]) @ W.
- Per 128-edge chunk (dst-tile sorted): payload rows are fetched with one
  indirect DMA (128 offsets, one per partition) and accumulated into the
  dst tile with a one-hot matmul in PSUM. Pad slots use loc7=-1 (one-hot
  never matches -> adds zero).
- Word pairs: same scheme into 20 item tiles; the matmul rhs carries a
  ones column so item counts fall out of the same PSUM accumulation.
- Score pairs p belong to core p//1024; x2 rows are routed via AllToAll.
"""
import sys
for p in ("/opt/trn_rl_repo", "/root/.axon_site/_ro/trn_rl_repo"):
    if p not in sys.path:
        sys.path.insert(0, p)
import numpy as np

NC = 8
NUM_USER = 50000
NUM_ITEM = 20000
DIM = 64
WDIM = 128
UPC = 6250
IPC = 2500
UPAD = 6272
BLOCK = 8832
NT_N = 69            # node tiles per core
ITEM_TILE0 = 49
R_T = 2560
NT_W = 20            # item tiles per core
NROW = NC * BLOCK    # 70656 xn rows
B = 8192
BPC = 1024
CELL = 384
SW = NC * CELL // 128  # send gather chunks (24)
K_E = 16             # chunks per edge offset/onehot group
K_W = 8              # chunks per word offset/onehot group
SLOPE = 0.01
SAMPLE_FILTER = True

_CACHE = {}


# ---------------------------------------------------------------- CPU prep

def _node_owner_local(v):
    v = np.asarray(v)
    is_user = v < NUM_USER
    c_u = v // UPC
    l_u = v - c_u * UPC
    i = v - NUM_USER
    c_i = i // IPC
    l_i = UPAD + (i - c_i * IPC)
    return (np.where(is_user, c_u, c_i).astype(np.int64),
            np.where(is_user, l_u, l_i).astype(np.int64))


def _relab_perm():
    perm = np.full(NROW, -1, np.int64)
    for c in range(NC):
        perm[c * BLOCK: c * BLOCK + UPC] = np.arange(c * UPC, (c + 1) * UPC)
        perm[c * BLOCK + UPAD: c * BLOCK + UPAD + IPC] = (
            NUM_USER + np.arange(c * IPC, (c + 1) * IPC))
    return perm


def _chunk_schedule(rows_per_core, loc_per_core, n_tiles, K):
    counts = np.zeros((NC, n_tiles), np.int64)
    srt = []
    for c in range(NC):
        order = np.argsort(loc_per_core[c], kind="stable")
        r, l = rows_per_core[c][order], loc_per_core[c][order]
        srt.append((r, l))
        counts[c] = np.bincount(l >> 7, minlength=n_tiles)
    n_chunks = np.maximum(np.ceil(counts / 128).astype(np.int64).max(0), 1)
    NCH = int(n_chunks.sum())
    NCHp = int(np.ceil(NCH / K) * K)
    cpt = n_chunks.copy()
    cpt[-1] += NCHp - NCH
    offs = np.zeros((NC, NCHp, 128), np.int32)
    loc7 = np.full((NC, NCHp, 128), -1.0, np.float32)
    for c in range(NC):
        r, l = srt[c]
        tiles = l >> 7
        start = np.searchsorted(tiles, np.arange(n_tiles))
        end = np.searchsorted(tiles, np.arange(n_tiles), side="right")
        ch0 = 0
        for t in range(n_tiles):
            nt = int(n_chunks[t])
            cnt = end[t] - start[t]
            fo = np.zeros(nt * 128, np.int32)
            fl = np.full(nt * 128, -1.0, np.float32)
            fo[:cnt] = r[start[t]:end[t]]
            fl[:cnt] = (l[start[t]:end[t]] - t * 128).astype(np.float32)
            offs[c, ch0:ch0 + nt] = fo.reshape(nt, 128)
            loc7[c, ch0:ch0 + nt] = fl.reshape(nt, 128)
            ch0 += nt
    # group chunks into instruction tiles [NG, 128, K]
    NG = NCHp // K
    g_o = np.ascontiguousarray(offs.reshape(NC, NG, K, 128).transpose(0, 1, 3, 2))
    g_l = np.ascontiguousarray(loc7.reshape(NC, NG, K, 128).transpose(0, 1, 3, 2))
    return cpt, g_o, g_l


def _prep(inputs):
    edge_index = np.asarray(inputs["edge_index"])
    words_tensor = np.asarray(inputs["words_tensor"])
    user_nodes = np.asarray(inputs["user_nodes"]).astype(np.int64)
    item_nodes = np.asarray(inputs["item_nodes"]).astype(np.int64)

    src, dst = edge_index[0].astype(np.int64), edge_index[1].astype(np.int64)
    items_w = words_tensor[0].astype(np.int64)
    words_w = words_tensor[1].astype(np.int64)

    if SAMPLE_FILTER:
        samp = np.zeros(NUM_USER + NUM_ITEM, bool)
        samp[user_nodes] = True
        samp[item_nodes] = True
        keep = samp[dst]
        src, dst = src[keep], dst[keep]
        samp_i = np.zeros(NUM_ITEM, bool)
        ii = item_nodes - NUM_USER
        samp_i[ii[item_nodes >= NUM_USER]] = True
        keepw = samp_i[items_w]
        items_w, words_w = items_w[keepw], words_w[keepw]

    so, sl = _node_owner_local(src)
    do, dl = _node_owner_local(dst)
    grow = (so * BLOCK + sl)
    e_rows = [grow[do == c] for c in range(NC)]
    e_locs = [dl[do == c] for c in range(NC)]
    cpt_e, e_offs, e_loc7 = _chunk_schedule(e_rows, e_locs, NT_N, K_E)

    owner_w = items_w // IPC
    w_rows = [words_w[owner_w == c] for c in range(NC)]
    w_locs = [(items_w - owner_w * IPC)[owner_w == c] for c in range(NC)]
    cpt_w, w_offs, w_loc7 = _chunk_schedule(w_rows, w_locs, NT_W, K_W)

    # score routing
    uo, ul = _node_owner_local(user_nodes)
    io_, il = _node_owner_local(item_nodes)
    dest = np.arange(B) // BPC
    fill = np.zeros((NC, NC), np.int64)
    send_rows = np.zeros((NC, NC * CELL), np.int64)
    recv_pos_u = np.empty(B, np.int64)
    recv_pos_i = np.empty(B, np.int64)
    for p in range(B):
        d = dest[p]
        for kind, (s, l) in enumerate(((uo[p], ul[p]), (io_[p], il[p]))):
            slot = fill[s][d]
            assert slot < CELL, "a2a cell overflow"
            fill[s][d] += 1
            send_rows[s][d * CELL + slot] = l
            if kind == 0:
                recv_pos_u[p] = s * CELL + slot
            else:
                recv_pos_i[p] = s * CELL + slot
    send_offs = np.zeros((NC, 128, SW), np.int32)
    j = np.arange(NC * CELL)
    for c in range(NC):
        send_offs[c, j % 128, j // 128] = send_rows[c]
    recv_offs = np.zeros((NC, 128, 16), np.int32)
    q = np.arange(BPC)
    for c in range(NC):
        mine = slice(c * BPC, (c + 1) * BPC)
        recv_offs[c, q % 128, q // 128] = recv_pos_u[mine]
        recv_offs[c, q % 128, 8 + q // 128] = recv_pos_i[mine]

    # permuted embeddings + per-core v_feat
    perm = _relab_perm()
    id_relab = np.zeros((NROW, DIM), np.float32)
    v = perm >= 0
    id_relab[v] = np.asarray(inputs["id_embedding"], np.float32)[perm[v]]
    v_feat = np.asarray(inputs["v_feat"], np.float32)
    vf = np.zeros((NC, R_T, WDIM), np.float32)
    for c in range(NC):
        vf[c, :IPC] = v_feat[c * IPC:(c + 1) * IPC]

    return dict(cpt_e=cpt_e, e_offs=e_offs, e_loc7=e_loc7,
                cpt_w=cpt_w, w_offs=w_offs, w_loc7=w_loc7,
                send_offs=send_offs, recv_offs=recv_offs,
                id_relab=id_relab, vf=vf)


# ------------------------------------------------------------- bass program

def _build_program(cpt_e, cpt_w, NGE, NGW):
    from concourse import bass, bacc, mybir
    import concourse.tile as tile
    dt = mybir.dt

    nc = bacc.Bacc(None, target_bir_lowering=False)
    f32 = dt.float32

    id_in = nc.dram_tensor("id_relab", [NROW, DIM], f32, kind="ExternalInput")
    wt_in = nc.dram_tensor("word_table", [100000, WDIM], f32, kind="ExternalInput")
    vf_in = nc.dram_tensor("vf", [R_T, WDIM], f32, kind="ExternalInput")
    eoff_in = nc.dram_tensor("e_offs", [NGE, 128, K_E], dt.int32, kind="ExternalInput")
    eloc_in = nc.dram_tensor("e_loc7", [NGE, 128, K_E], f32, kind="ExternalInput")
    woff_in = nc.dram_tensor("w_offs", [NGW, 128, K_W], dt.int32, kind="ExternalInput")
    wloc_in = nc.dram_tensor("w_loc7", [NGW, 128, K_W], f32, kind="ExternalInput")
    soff_in = nc.dram_tensor("send_offs", [128, SW], dt.int32, kind="ExternalInput")
    roff_in = nc.dram_tensor("recv_offs", [128, 16], dt.int32, kind="ExternalInput")
    cw_in = nc.dram_tensor("conv_weight", [DIM, DIM], f32, kind="ExternalInput")
    ww_in = nc.dram_tensor("weight_W", [DIM, DIM], f32, kind="ExternalInput")
    w2_in = nc.dram_tensor("weight_2", [DIM, DIM], f32, kind="ExternalInput")
    lw_in = nc.dram_tensor("lin_w", [256, DIM], f32, kind="ExternalInput")
    lb_in = nc.dram_tensor("lin_b_rep", [128, DIM], f32, kind="ExternalInput")
    iota_in = nc.dram_tensor("iota", [128, 128], f32, kind="ExternalInput")
    ident_in = nc.dram_tensor("ident", [128, 128], f32, kind="ExternalInput")

    xn_dram = nc.dram_tensor("xn", [NROW, DIM], f32)
    x2_dram = nc.dram_tensor("x2", [BLOCK, DIM], f32)
    out = nc.dram_tensor("scores_w", [128, 8], f32, kind="ExternalOutput")

    # edge chunk -> (group, col, tile, start, stop)
    def sched(cpt, K):
        s = []
        ch = 0
        for t, n in enumerate(cpt):
            for j in range(int(n)):
                s.append((ch // K, ch % K, t, j == 0, j == int(n) - 1))
                ch += 1
        return s

    esched = sched(cpt_e, K_E)
    wsched = sched(cpt_w, K_W)

    with tile.TileContext(nc) as tc:
        with tc.tile_pool(name="const", bufs=1) as cpool, \
             tc.tile_pool(name="persist", bufs=1) as pp, \
             tc.tile_pool(name="work", bufs=4) as wp, \
             tc.tile_pool(name="psum_e", bufs=4, space="PSUM") as pse, \
             tc.tile_pool(name="psum_w", bufs=2, space="PSUM") as psw, \
             tc.tile_pool(name="psum_m", bufs=1, space="PSUM") as psm, \
             tc.tile_pool(name="dram", bufs=1, space="DRAM") as dpool:

            iota = cpool.tile([128, 128], f32)
            ident = cpool.tile([128, 128], f32)
            cw = cpool.tile([DIM, DIM], f32)
            ww = cpool.tile([DIM, DIM], f32)
            w2 = cpool.tile([DIM, DIM], f32)
            lw = cpool.tile([128, 2 * DIM], f32)   # lin_w as two [128,64] halves
            lb = cpool.tile([128, DIM], f32)
            nc.sync.dma_start(out=iota[:], in_=iota_in[:])
            nc.sync.dma_start(out=ident[:], in_=ident_in[:])
            nc.sync.dma_start(out=cw[:], in_=cw_in[:])
            nc.sync.dma_start(out=ww[:], in_=ww_in[:])
            nc.sync.dma_start(out=w2[:], in_=w2_in[:])
            nc.sync.dma_start(out=lw[:, 0:DIM], in_=lw_in[0:128, :])
            nc.sync.dma_start(out=lw[:, DIM:2 * DIM], in_=lw_in[128:256, :])
            nc.sync.dma_start(out=lb[:], in_=lb_in[:])

            tf_sb = pp.tile([128, NT_W * WDIM], f32)
            fh_sb = pp.tile([128, NT_W * DIM], f32)
            pg_sb = pp.tile([128, NT_N * DIM], f32)
            x2_sb = pp.tile([128, NT_N * DIM], f32)

            # ---- phase N: normalize id_relab -> xn_dram (p-outer layout)
            NTT = NROW // 128          # 552 rows per partition
            NCHN = 8
            CH = NTT // NCHN           # 69 per chunk
            vi = id_in[:, :].rearrange("(p t) d -> p t d", p=128)
            vo = xn_dram[:, :].rearrange("(p t) d -> p t d", p=128)
            with tc.tile_pool(name="npool", bufs=1) as npool:
                for cch in range(NCHN):
                    x = npool.tile([128, CH * DIM], f32, tag="nx")
                    sq = npool.tile([128, CH * DIM], f32, tag="nsq")
                    ss = npool.tile([128, CH], f32, tag="nss")
                    x3 = x[:].rearrange("p (t d) -> p t d", d=DIM)
                    sq3 = sq[:].rearrange("p (t d) -> p t d", d=DIM)
                    nc.sync.dma_start(out=x3, in_=vi[:, cch * CH:(cch + 1) * CH, :])
                    nc.vector.tensor_tensor(out=sq3, in0=x3, in1=x3,
                                            op=mybir.AluOpType.mult)
                    nc.vector.reduce_sum(out=ss[:], in_=sq3,
                                         axis=mybir.AxisListType.X)
                    nc.scalar.sqrt(ss[:], ss[:])
                    nc.vector.tensor_scalar_max(out=ss[:], in0=ss[:], scalar1=1e-12)
                    nc.vector.reciprocal(ss[:], ss[:])
                    nc.vector.tensor_tensor(
                        out=x3, in0=x3,
                        in1=ss[:][:, :, None].to_broadcast([128, CH, DIM]),
                        op=mybir.AluOpType.mult)
                    nc.sync.dma_start(out=vo[:, cch * CH:(cch + 1) * CH, :], in_=x3)

            # ---- phase W: word aggregation into tf_sb
            wpsum = None
            for gi in range(NGW):
                woff = wp.tile([128, K_W], dt.int32, tag="woff")
                wloc = wp.tile([128, K_W], f32, tag="wloc")
                wpay = wp.tile([128, K_W * (WDIM + 1)], f32, tag="wpay")
                woh = wp.tile([128, K_W * 128], f32, tag="woh")
                nc.sync.dma_start(out=woff[:], in_=woff_in[gi])
                nc.sync.dma_start(out=wloc[:], in_=wloc_in[gi])
                pay3 = wpay[:].rearrange("p (k d) -> p k d", d=WDIM + 1)
                nc.vector.memset(pay3[:, :, WDIM:WDIM + 1], 1.0)
                oh3 = woh[:].rearrange("p (k d) -> p k d", d=128)
                nc.vector.tensor_tensor(
                    out=oh3,
                    in0=wloc[:][:, :, None].to_broadcast([128, K_W, 128]),
                    in1=iota[:][:, None, :].to_broadcast([128, K_W, 128]),
                    op=mybir.AluOpType.is_equal)
                for k in range(K_W):
                    ci = gi * K_W + k
                    if ci >= len(wsched):
                        break
                    _, _, t, st, sp = wsched[ci]
                    nc.gpsimd.indirect_dma_start(
                        out=pay3[:, k, 0:WDIM], out_offset=None,
                        in_=wt_in[:, :],
                        in_offset=bass.IndirectOffsetOnAxis(ap=woff[:, k:k + 1], axis=0))
                    if st:
                        wpsum = psw.tile([128, WDIM + 1], f32, tag="wps")
                    nc.tensor.matmul(out=wpsum[:], lhsT=oh3[:, k, :],
                                     rhs=pay3[:, k, :], start=st, stop=sp)
                    if sp:
                        rec = wp.tile([128, 1], f32, tag="wrec")
                        nc.vector.tensor_scalar_max(out=rec[:], in0=wpsum[:, WDIM:WDIM + 1], scalar1=1.0)
                        nc.vector.reciprocal(rec[:], rec[:])
                        nc.vector.tensor_tensor(
                            out=tf_sb[:, t * WDIM:(t + 1) * WDIM],
                            in0=wpsum[:, 0:WDIM],
                            in1=rec[:].to_broadcast([128, WDIM]),
                            op=mybir.AluOpType.mult)

            # ---- phase V: item pipeline -> fh_sb
            vf_sb = pp.tile([128, NT_W * WDIM], f32)
            nc.sync.dma_start(
                out=vf_sb[:].rearrange("p (t d) -> p t d", d=WDIM),
                in_=vf_in[:, :].rearrange("(t p) d -> p t d", p=128))
            for t in range(NT_W):
                ps_t = psm.tile([128, 128], f32, tag="tr")
                nc.tensor.transpose(out=ps_t[:], in_=vf_sb[:, t * WDIM:(t + 1) * WDIM],
                                    identity=ident[:])
                vT = wp.tile([128, 128], f32, tag="vT")
                nc.scalar.copy(out=vT[:], in_=ps_t[:])
                ps_t2 = psm.tile([128, 128], f32, tag="tr")
                nc.tensor.transpose(out=ps_t2[:], in_=tf_sb[:, t * WDIM:(t + 1) * WDIM],
                                    identity=ident[:])
                tT = wp.tile([128, 128], f32, tag="tT")
                nc.scalar.copy(out=tT[:], in_=ps_t2[:])
                fps = psm.tile([128, DIM], f32, tag="mm")
                nc.tensor.matmul(out=fps[:], lhsT=vT[:], rhs=lw[:, 0:DIM],
                                 start=True, stop=False)
                nc.tensor.matmul(out=fps[:], lhsT=tT[:], rhs=lw[:, DIM:2 * DIM],
                                 start=False, stop=True)
                fsum = wp.tile([128, DIM], f32, tag="fsum")
                nc.vector.tensor_add(out=fsum[:], in0=fps[:], in1=lb[:])
                f_sb = wp.tile([128, DIM], f32, tag="fsb")
                nc.scalar.activation(f_sb[:], fsum[:],
                                     mybir.ActivationFunctionType.Lrelu, alpha=SLOPE)
                ps_t3 = psm.tile([128, 128], f32, tag="tr")
                nc.tensor.transpose(out=ps_t3[0:64, :], in_=f_sb[:],
                                    identity=ident[:])
                fT = wp.tile([64, 128], f32, tag="fT")
                nc.scalar.copy(out=fT[:], in_=ps_t3[0:64, :])
                fhp = psm.tile([128, DIM], f32, tag="mm")
                nc.tensor.matmul(out=fhp[:], lhsT=fT[:], rhs=w2[:],
                                 start=True, stop=True)
                nc.scalar.copy(out=fh_sb[:, t * DIM:(t + 1) * DIM], in_=fhp[:])

            # ---- phase E: edge aggregation into pg_sb
            epsum = None
            for gi in range(NGE):
                eoff = wp.tile([128, K_E], dt.int32, tag="eoff")
                eloc = wp.tile([128, K_E], f32, tag="eloc")
                epay = wp.tile([128, K_E * DIM], f32, tag="epay")
                eoh = wp.tile([128, K_E * 128], f32, tag="eoh")
                nc.sync.dma_start(out=eoff[:], in_=eoff_in[gi])
                nc.sync.dma_start(out=eloc[:], in_=eloc_in[gi])
                pay3 = epay[:].rearrange("p (k d) -> p k d", d=DIM)
                oh3 = eoh[:].rearrange("p (k d) -> p k d", d=128)
                nc.vector.tensor_tensor(
                    out=oh3,
                    in0=eloc[:][:, :, None].to_broadcast([128, K_E, 128]),
                    in1=iota[:][:, None, :].to_broadcast([128, K_E, 128]),
                    op=mybir.AluOpType.is_equal)
                for k in range(K_E):
                    ci = gi * K_E + k
                    if ci >= len(esched):
                        break
                    _, _, t, st, sp = esched[ci]
                    nc.gpsimd.indirect_dma_start(
                        out=pay3[:, k, :], out_offset=None,
                        in_=xn_dram[:, :],
                        in_offset=bass.IndirectOffsetOnAxis(ap=eoff[:, k:k + 1], axis=0))
                    if st:
                        epsum = pse.tile([128, DIM], f32, tag="eps")
                    nc.tensor.matmul(out=epsum[:], lhsT=oh3[:, k, :],
                                     rhs=pay3[:, k, :], start=st, stop=sp)
                    if sp:
                        nc.scalar.copy(out=pg_sb[:, t * DIM:(t + 1) * DIM],
                                       in_=epsum[:])

            # ---- phase X: node tail -> x2_sb -> x2_dram
            for t in range(NT_N):
                ps_t = psm.tile([128, 128], f32, tag="tr")
                nc.tensor.transpose(out=ps_t[0:64, :],
                                    in_=pg_sb[:, t * DIM:(t + 1) * DIM],
                                    identity=ident[:])
                pgT = wp.tile([64, 128], f32, tag="pgT")
                nc.scalar.copy(out=pgT[:], in_=ps_t[0:64, :])
                x1p = psm.tile([128, DIM], f32, tag="mm")
                nc.tensor.matmul(out=x1p[:], lhsT=pgT[:], rhs=cw[:],
                                 start=True, stop=True)
                x1_sb = wp.tile([128, DIM], f32, tag="x1")
                nc.scalar.activation(x1_sb[:], x1p[:],
                                     mybir.ActivationFunctionType.Lrelu, alpha=SLOPE)
                ps_t2 = psm.tile([128, 128], f32, tag="tr")
                nc.tensor.transpose(out=ps_t2[0:64, :], in_=x1_sb[:],
                                    identity=ident[:])
                x1T = wp.tile([64, 128], f32, tag="x1T")
                nc.scalar.copy(out=x1T[:], in_=ps_t2[0:64, :])
                x2p = psm.tile([128, DIM], f32, tag="mm")
                nc.tensor.matmul(out=x2p[:], lhsT=x1T[:], rhs=ww[:],
                                 start=True, stop=True)
                if t >= ITEM_TILE0:
                    xsum = wp.tile([128, DIM], f32, tag="xsum")
                    nc.vector.tensor_add(
                        out=xsum[:], in0=x2p[:],
                        in1=fh_sb[:, (t - ITEM_TILE0) * DIM:(t - ITEM_TILE0 + 1) * DIM])
                    nc.scalar.activation(x2_sb[:, t * DIM:(t + 1) * DIM], xsum[:],
                                         mybir.ActivationFunctionType.Lrelu, alpha=SLOPE)
                else:
                    nc.scalar.activation(x2_sb[:, t * DIM:(t + 1) * DIM], x2p[:],
                                         mybir.ActivationFunctionType.Lrelu, alpha=SLOPE)
            nc.sync.dma_start(
                out=x2_dram[:, :].rearrange("(t p) d -> p t d", p=128),
                in_=x2_sb[:].rearrange("p (t d) -> p t d", d=DIM))

            # ---- phase S: score routing + dots
            soff = pp.tile([128, SW], dt.int32)
            roff = pp.tile([128, 16], dt.int32)
            nc.sync.dma_start(out=soff[:], in_=soff_in[:])
            nc.sync.dma_start(out=roff[:], in_=roff_in[:])
            send_sb = pp.tile([128, SW * DIM], f32)
            s3 = send_sb[:].rearrange("p (k d) -> p k d", d=DIM)
            for k in range(SW):
                nc.gpsimd.indirect_dma_start(
                    out=s3[:, k, :], out_offset=None, in_=x2_dram[:, :],
                    in_offset=bass.IndirectOffsetOnAxis(ap=soff[:, k:k + 1], axis=0))
            a2a_in = dpool.tile([NC, CELL * DIM], f32)
            a2a_out = dpool.tile([NC, CELL * DIM], f32)
            nc.sync.dma_start(
                out=a2a_in[:].rearrange("c (s p d) -> p (c s) d", p=128, d=DIM),
                in_=s3)
            nc.gpsimd.collective_compute(
                "AllToAll", mybir.AluOpType.bypass,
                replica_groups=[list(range(NC))],
                ins=[a2a_in.opt()], outs=[a2a_out.opt()])
            recv_flat = a2a_out[:].rearrange("c (r d) -> (c r) d", d=DIM)
            pairs = pp.tile([128, 16 * DIM], f32)
            p3 = pairs[:].rearrange("p (k d) -> p k d", d=DIM)
            for k in range(16):
                nc.gpsimd.indirect_dma_start(
                    out=p3[:, k, :], out_offset=None, in_=recv_flat,
                    in_offset=bass.IndirectOffsetOnAxis(ap=roff[:, k:k + 1], axis=0))
            prod = pp.tile([128, 8 * DIM], f32)
            pr3 = prod[:].rearrange("p (k d) -> p k d", d=DIM)
            nc.vector.tensor_tensor(out=pr3, in0=p3[:, 0:8, :], in1=p3[:, 8:16, :],
                                    op=mybir.AluOpType.mult)
            sc = pp.tile([128, 8], f32)
            nc.vector.reduce_sum(out=sc[:], in_=pr3, axis=mybir.AxisListType.X)
            nc.sync.dma_start(out=out[:], in_=sc[:])

    nc.finalize()
    return nc


# ------------------------------------------------------------------- kernel

def kernel(**inputs):
    from concourse.bass_utils import run_bass_kernel_spmd

    pr = _prep(inputs)
    NGE = pr["e_offs"].shape[1]
    NGW = pr["w_offs"].shape[1]
    key = (tuple(pr["cpt_e"]), tuple(pr["cpt_w"]), NGE, NGW)
    if key not in _CACHE:
        _CACHE[key] = _build_program(pr["cpt_e"], pr["cpt_w"], NGE, NGW)
    nc = _CACHE[key]

    iota = np.broadcast_to(np.arange(128, dtype=np.float32), (128, 128)).copy()
    ident = np.eye(128, dtype=np.float32)
    lin_b_rep = np.broadcast_to(np.asarray(inputs["lin_b"], np.float32), (128, DIM)).copy()

    in_maps = []
    for c in range(NC):
        in_maps.append({
            "id_relab": pr["id_relab"],
            "word_table": np.asarray(inputs["word_table"], np.float32),
            "vf": pr["vf"][c],
            "e_offs": pr["e_offs"][c],
            "e_loc7": pr["e_loc7"][c],
            "w_offs": pr["w_offs"][c],
            "w_loc7": pr["w_loc7"][c],
            "send_offs": pr["send_offs"][c],
            "recv_offs": pr["recv_offs"][c],
            "conv_weight": np.asarray(inputs["conv_weight"], np.float32),
            "weight_W": np.asarray(inputs["weight_W"], np.float32),
            "weight_2": np.asarray(inputs["weight_2"], np.float32),
            "lin_w": np.asarray(inputs["lin_w"], np.float32),
            "lin_b_rep": lin_b_rep,
            "iota": iota,
            "ident": ident,
        })
    res = run_bass_kernel_spmd(nc, in_maps, list(range(NC)))
    scores = np.empty(B, np.float32)
    for c in range(NC):
        w = res.results[c]["scores_w"]           # [128, 8]
        scores[c * BPC:(c + 1) * BPC] = w.T.ravel()
    return scores


kernel.run_traced = None  # set by test harness if needed
